# revision 31
# baseline (speedup 1.0000x reference)
"""Trainium2 Bass kernel for nn_DivrocLoss (trilinear splat histogram + Huber loss).

Strategy (8 NeuronCores, SPMD):
  - Spatial sharding over the 256-slab z axis: core c owns slabs [32c, 32c+32).
    Each (point, volume) pair becomes ONE record carrying its grid-space y, x
    coords and BOTH z-tap weights (w0 = 1-fz for slab z0, w1 = fz for slab
    z0+1); records are binned by (z0 slab -> core, y-half, x-half, volume).
    Records whose z-taps straddle a core boundary are split into two
    single-tap records. Boundary slabs receive the split halves, so their
    bins get a larger static cap (NB0) than interior slabs (NBI).
  - On device, each core processes z0-groups in slab order with rotating PSUM
    banks: a batch of 128 records builds its (negated) y-tent and x-tent
    bf16 [128,128] tiles ONCE (DVE iota-subtract + ACT Abs + DVE sub/min),
    then two weighted stationaries a0 = tentY*w0, a1 = tentY*w1 feed two PE
    matmuls accumulating into slab s and slab s+1 banks. Window-straddling
    y/x tap pairs are duplicated into both windows by the host; the
    window-local tents pick up exactly the in-window taps and out-of-grid
    taps vanish (grid_sample zero-padding semantics).
  - pred and gt accumulate in separate PSUM banks; slab evacuation computes
    d = pred - gt and fused Huber partial sums
    Huber(d) = 0.5*|d|^2 - 0.5*relu(|d|-1)^2 via activation accumulate.
  - Host sums the 8 cores' [128, 64] partial-sum tiles.
"""

import sys

sys.path.insert(0, "/opt/trn_rl_repo")

import numpy as np
import ml_dtypes

from concourse import bacc, bass, mybir
import concourse.tile as tile
from concourse.bass_utils import run_bass_kernel_spmd

GRID = 256
CORES = 8
SLABS = 32  # slabs per core
YHALVES = 2
XHALVES = 2
VOLS = 2  # pred / gt accumulate into separate PSUM banks
NQ = YHALVES * XHALVES * VOLS  # 8 (yh, xh, vol) combos per slab position
NB0 = 35  # batches per group at slab position 0 (receives straddle halves)
NBI = 19  # batches per group at interior slab positions
NBS = [NB0] + [NBI] * (SLABS - 1)
TOT = NQ * sum(NBS)  # total batch-columns per core (4992)

F32 = mybir.dt.float32
BF16 = mybir.dt.bfloat16


def _group_col_offsets():
    offs = np.zeros(SLABS * NQ, dtype=np.int64)
    col = 0
    for s in range(SLABS):
        for q in range(NQ):
            offs[s * NQ + q] = col
            col += NBS[s]
    assert col == TOT
    return offs


def _prepare_shards(registration_pred, registration_gt, coords):
    """Build per-core [128, TOT] f32 arrays Y, X, W0, W1 of z-pair records."""
    ys, xs, w0s, w1s, bins = [], [], [], [], []
    for vol, reg in ((0, registration_pred), (1, registration_gt)):
        p = coords.astype(np.float32) + reg.astype(np.float32)
        # mirror the reference's exact f32 expression ((g+1)*size - 1) * 0.5
        g = ((p + np.float32(1.0)) * np.float32(GRID) - np.float32(1.0)) * np.float32(
            0.5
        )
        gx = g[:, 0]
        gy = g[:, 1]
        gz = g[:, 2]
        z0 = np.floor(gz)
        fz = (gz - z0).astype(np.float32)
        z0 = z0.astype(np.int64)
        w0 = (1.0 - fz).astype(np.float32)
        w1 = fz.copy()
        # z0 == -1: only the z=0 tap is valid -> shift record to z0=0
        shift = z0 == -1
        w0 = np.where(shift, fz, w0)
        w1 = np.where(shift, 0.0, w1)
        z0 = np.where(shift, 0, z0)
        # z0 == 255: the z=256 tap is out of grid
        w1 = np.where(z0 == GRID - 1, 0.0, w1)
        keep = (z0 >= 0) & (z0 <= GRID - 1)
        z0k, gyk, gxk = z0[keep], gy[keep], gx[keep]
        w0k, w1k = w0[keep], w1[keep]
        # split records whose taps straddle a core boundary
        strad = ((z0k % SLABS) == SLABS - 1) & (z0k < GRID - 1)
        w1a = np.where(strad, 0.0, w1k)
        recs = [
            (z0k, gyk, gxk, w0k, w1a),
            (z0k[strad] + 1, gyk[strad], gxk[strad], w1k[strad], np.zeros(strad.sum(), np.float32)),
        ]
        for zz, gyv, gxv, rw0, rw1 in recs:
            y0 = np.floor(gyv)
            x0 = np.floor(gxv)
            yh = np.clip(y0 // 128, 0, 1).astype(np.int64)
            xh = np.clip(x0 // 128, 0, 1).astype(np.int64)
            # duplicate window-straddling y/x tap pairs into the upper window
            dupy = y0 == 127
            dupx = x0 == 127
            dupyx = dupy & dupx
            for sel, byh, bxh in (
                (slice(None), yh, xh),
                (dupy, 1, xh[dupy]),
                (dupx, yh[dupx], 1),
                (dupyx, 1, 1),
            ):
                ys.append(gyv[sel])
                xs.append(gxv[sel])
                w0s.append(rw0[sel])
                w1s.append(rw1[sel])
                bins.append(((zz[sel] * 2 + byh) * 2 + bxh) * 2 + vol)
    Y = np.concatenate(ys)
    X = np.concatenate(xs)
    W0 = np.concatenate(w0s)
    W1 = np.concatenate(w1s)
    B = np.concatenate(bins)  # global bin in [0, 2048)

    order = np.argsort(B, kind="stable")
    Y, X, W0, W1, B = Y[order], X[order], W0[order], W1[order], B[order]
    nbins = GRID * NQ // SLABS * SLABS  # 2048
    nbins = GRID * NQ
    counts = np.bincount(B, minlength=nbins)

    offs_core = _group_col_offsets()  # per (slab_pos, q) within-core col offset
    zz = np.arange(GRID)
    core_of = zz // SLABS
    pos_of = zz % SLABS
    bin_caps = np.repeat(np.array(NBS)[pos_of] * 128, NQ)
    if (counts > bin_caps).any():
        raise RuntimeError("bin overflow")
    # global column offset per bin
    bin_cols = (
        core_of.repeat(NQ) * TOT
        + offs_core[(pos_of.repeat(NQ) * NQ) + np.tile(np.arange(NQ), GRID)]
    )

    starts = np.zeros(nbins + 1, dtype=np.int64)
    np.cumsum(counts, out=starts[1:])
    rank = np.arange(len(B), dtype=np.int64) - starts[B]
    dest = bin_cols[B] * 128 + rank

    def field_tiles(vals):
        flat = np.zeros(CORES * TOT * 128, dtype=np.float32)
        flat[dest] = vals
        out = []
        for c in range(CORES):
            block = flat[c * TOT * 128 : (c + 1) * TOT * 128]
            out.append(np.ascontiguousarray(block.reshape(TOT, 128).T))
        return out

    return list(
        zip(field_tiles(Y), field_tiles(X), field_tiles(W0), field_tiles(W1))
    )


def _sb_chunks(nb):
    """Split nb batches into superblocks of up to 8."""
    out = []
    i = 0
    while i < nb:
        sz = min(8, nb - i)
        out.append((i, sz))
        i += sz
    return out


def _build_program():
    nc = bacc.Bacc("TRN2", target_bir_lowering=False, debug=False, num_devices=CORES)
    Yd = nc.declare_dram_parameter("Y", [128, TOT], F32, isOutput=False)
    Xd = nc.declare_dram_parameter("X", [128, TOT], F32, isOutput=False)
    W0d = nc.declare_dram_parameter("W0", [128, TOT], F32, isOutput=False)
    W1d = nc.declare_dram_parameter("W1", [128, TOT], F32, isOutput=False)
    IOTAd = nc.declare_dram_parameter("IOTA", [128, GRID], BF16, isOutput=False)
    OUTd = nc.declare_dram_parameter("OUT", [128, 2 * SLABS], F32, isOutput=True)

    AluOp = mybir.AluOpType
    Act = mybir.ActivationFunctionType
    offs_core = _group_col_offsets()

    with tile.TileContext(nc) as tc:
        with (
            tc.tile_pool(name="persist", bufs=1) as persist,
            tc.tile_pool(name="eab", bufs=4) as eab,
            tc.tile_pool(name="tab", bufs=4) as tab,
            tc.tile_pool(name="atile", bufs=8) as atile,
            tc.tile_pool(name="evac", bufs=2) as evac,
            tc.tile_pool(name="psum", bufs=8, space="PSUM") as psum,
        ):
            y_t = persist.tile([128, TOT], F32, tag="yt")
            nc.sync.dma_start(out=y_t[:], in_=Yd[:])
            x_t = persist.tile([128, TOT], F32, tag="xt")
            nc.sync.dma_start(out=x_t[:], in_=Xd[:])
            w0_t = persist.tile([128, TOT], F32, tag="w0t")
            nc.sync.dma_start(out=w0_t[:], in_=W0d[:])
            w1_t = persist.tile([128, TOT], F32, tag="w1t")
            nc.sync.dma_start(out=w1_t[:], in_=W1d[:])
            iota_t = persist.tile([128, GRID], BF16, tag="iota")
            nc.sync.dma_start(out=iota_t[:], in_=IOTAd[:])
            acc_u = persist.tile([128, SLABS], F32, tag="accu")
            acc_r = persist.tile([128, SLABS], F32, tag="accr")
            negone = persist.tile([128, 1], F32, tag="negone")
            nc.gpsimd.memset(negone[:], -1.0)

            # Each z0-group s writes two tile-pairs: cur (slab s, w0 taps) and
            # nxt (slab s+1, w1 taps). Each tile sees one contiguous PSUM
            # accumulation group; slab s's total = cur(s) + nxt from group
            # s-1, summed during evacuation.
            prev_p = None
            prev_g = None
            for s in range(SLABS):
                cur_p = psum.tile([128, 512], F32, tag="bank")
                cur_g = psum.tile([128, 512], F32, tag="bank")
                nxt_p = psum.tile([128, 512], F32, tag="bank")
                nxt_g = psum.tile([128, 512], F32, tag="bank")
                cur = [cur_p, cur_g]
                nxt = [nxt_p, nxt_g]
                nb = NBS[s]
                for gy in range(YHALVES):
                    for gx in range(XHALVES):
                        qq = gy * XHALVES + gx
                        for vol in range(VOLS):
                            q = qq * VOLS + vol
                            base = offs_core[s * NQ + q]
                            cr = cur[vol][:, qq * 128 : (qq + 1) * 128]
                            nr = nxt[vol][:, qq * 128 : (qq + 1) * 128]
                            for sb0, sbn in _sb_chunks(nb):
                                da = eab.tile([128, 8 * 128], BF16, tag="da")
                                db = eab.tile([128, 8 * 128], BF16, tag="db")
                                for j in range(sbn):
                                    c = base + sb0 + j
                                    nc.vector.tensor_scalar(
                                        out=da[:, j * 128 : (j + 1) * 128],
                                        in0=iota_t[:, gy * 128 : gy * 128 + 128],
                                        scalar1=y_t[:, c : c + 1],
                                        scalar2=None,
                                        op0=AluOp.subtract,
                                    )
                                    nc.vector.tensor_scalar(
                                        out=db[:, j * 128 : (j + 1) * 128],
                                        in0=iota_t[:, gx * 128 : gx * 128 + 128],
                                        scalar1=x_t[:, c : c + 1],
                                        scalar2=None,
                                        op0=AluOp.subtract,
                                    )
                                ea = eab.tile([128, 8 * 128], BF16, tag="ea")
                                eb = eab.tile([128, 8 * 128], BF16, tag="eb")
                                nc.scalar.activation(
                                    out=ea[:, : sbn * 128],
                                    in_=da[:, : sbn * 128],
                                    func=Act.Abs,
                                    bias=0.0,
                                    scale=1.0,
                                )
                                nc.scalar.activation(
                                    out=eb[:, : sbn * 128],
                                    in_=db[:, : sbn * 128],
                                    func=Act.Abs,
                                    bias=0.0,
                                    scale=1.0,
                                )
                                ta = tab.tile([128, 8 * 128], BF16, tag="ta")
                                tb = tab.tile([128, 8 * 128], BF16, tag="tb")
                                # negated tents min(|t|-1, 0); negations cancel
                                nc.vector.tensor_scalar(
                                    out=ta[:, : sbn * 128],
                                    in0=ea[:, : sbn * 128],
                                    scalar1=1.0,
                                    scalar2=0.0,
                                    op0=AluOp.subtract,
                                    op1=AluOp.min,
                                )
                                nc.vector.tensor_scalar(
                                    out=tb[:, : sbn * 128],
                                    in0=eb[:, : sbn * 128],
                                    scalar1=1.0,
                                    scalar2=0.0,
                                    op0=AluOp.subtract,
                                    op1=AluOp.min,
                                )
                                for j in range(sbn):
                                    c = base + sb0 + j
                                    first = sb0 + j == 0
                                    last = sb0 + j == nb - 1
                                    a0 = atile.tile([128, 128], BF16, tag="a0")
                                    nc.vector.tensor_scalar(
                                        out=a0[:],
                                        in0=ta[:, j * 128 : (j + 1) * 128],
                                        scalar1=w0_t[:, c : c + 1],
                                        scalar2=None,
                                        op0=AluOp.mult,
                                    )
                                    nc.tensor.matmul(
                                        cr,
                                        a0[:],
                                        tb[:, j * 128 : (j + 1) * 128],
                                        start=first,
                                        stop=last,
                                    )
                                    a1 = atile.tile([128, 128], BF16, tag="a1")
                                    nc.vector.tensor_scalar(
                                        out=a1[:],
                                        in0=ta[:, j * 128 : (j + 1) * 128],
                                        scalar1=w1_t[:, c : c + 1],
                                        scalar2=None,
                                        op0=AluOp.mult,
                                    )
                                    nc.tensor.matmul(
                                        nr,
                                        a1[:],
                                        tb[:, j * 128 : (j + 1) * 128],
                                        start=first,
                                        stop=last,
                                    )
                # evacuate slab s: total = cur(s) + prev-group nxt; then Huber
                p_sb = evac.tile([128, 512], BF16, tag="psb")
                g_sb = evac.tile([128, 512], BF16, tag="gsb")
                if prev_p is None:
                    nc.vector.tensor_copy(out=p_sb[:], in_=cur_p[:])
                    nc.vector.tensor_copy(out=g_sb[:], in_=cur_g[:])
                else:
                    pp = evac.tile([128, 512], BF16, tag="pp")
                    nc.vector.tensor_copy(out=pp[:], in_=prev_p[:])
                    nc.vector.tensor_tensor(
                        out=p_sb[:], in0=cur_p[:], in1=pp[:], op=AluOp.add
                    )
                    gg = evac.tile([128, 512], BF16, tag="gg")
                    nc.vector.tensor_copy(out=gg[:], in_=prev_g[:])
                    nc.vector.tensor_tensor(
                        out=g_sb[:], in0=cur_g[:], in1=gg[:], op=AluOp.add
                    )
                d_sb = evac.tile([128, 512], BF16, tag="dsb")
                nc.vector.tensor_tensor(
                    out=d_sb[:], in0=p_sb[:], in1=g_sb[:], op=AluOp.subtract
                )
                u = evac.tile([128, 512], BF16, tag="u")
                nc.vector.scalar_tensor_tensor(
                    out=u[:],
                    in0=d_sb[:],
                    scalar=-1.0,
                    in1=d_sb[:],
                    op0=AluOp.mult,
                    op1=AluOp.max,
                )
                r = evac.tile([128, 512], BF16, tag="r")
                nc.scalar.activation(
                    out=r[:], in_=u[:], func=Act.Relu, bias=negone[:], scale=1.0
                )
                squ = evac.tile([128, 512], BF16, tag="squ")
                nc.scalar.activation(
                    out=squ[:],
                    in_=u[:],
                    func=Act.Square,
                    accum_out=acc_u[:, s : s + 1],
                )
                sqr = evac.tile([128, 512], BF16, tag="sqr")
                nc.scalar.activation(
                    out=sqr[:],
                    in_=r[:],
                    func=Act.Square,
                    accum_out=acc_r[:, s : s + 1],
                )
                prev_p, prev_g = nxt_p, nxt_g
            nc.sync.dma_start(out=OUTd[:, 0:SLABS], in_=acc_u[:])
            nc.sync.dma_start(out=OUTd[:, SLABS : 2 * SLABS], in_=acc_r[:])
    nc.compile()
    return nc


_PROGRAM_CACHE = {}


def _get_program():
    if "nc" not in _PROGRAM_CACHE:
        _PROGRAM_CACHE["nc"] = _build_program()
    return _PROGRAM_CACHE["nc"]


def _iota_input():
    return np.broadcast_to(
        np.arange(GRID, dtype=ml_dtypes.bfloat16)[None, :], (128, GRID)
    ).copy()


def kernel(registration_pred, registration_gt, coords, _trace=False):
    shards = _prepare_shards(registration_pred, registration_gt, coords)
    iota = _iota_input()
    nc = _get_program()
    in_maps = [
        {"Y": y, "X": x, "W0": w0, "W1": w1, "IOTA": iota}
        for (y, x, w0, w1) in shards
    ]
    res = run_bass_kernel_spmd(nc, in_maps, list(range(CORES)), trace=_trace)
    total = 0.0
    for r in res.results:
        out = r["OUT"].astype(np.float64)
        total += 0.5 * (out[:, :SLABS].sum() - out[:, SLABS:].sum())
    if _trace:
        kernel.last_exec_time_ns = res.exec_time_ns
        kernel.last_results = res
    return np.float32(total)


# revision 41
# speedup vs baseline: 1.2554x; 1.2554x over previous
"""Trainium2 Bass kernel for nn_DivrocLoss (trilinear splat histogram + Huber loss).

Strategy (8 NeuronCores, SPMD):
  - Spatial sharding over the 256-slab z axis: core c owns slabs [32c, 32c+32).
    Each (point, volume) pair becomes ONE record carrying its grid-space y, x
    coords and BOTH z-tap weights (w0 = 1-fz for slab z0, w1 = fz for slab
    z0+1, with the pred/gt sign folded into the weights); records are binned
    by (z0 slab -> core, y-half, x-half).
    Records whose z-taps straddle a core boundary are split into two
    single-tap records. Boundary slabs receive the split halves, so their
    bins get a larger static cap (NB0) than interior slabs (NBI).
  - On device, each core processes z0-groups in slab order with rotating PSUM
    banks: a batch of 128 records builds its (negated) y-tent and x-tent
    bf16 [128,128] tiles ONCE (DVE iota-subtract + ACT Abs + DVE sub/min),
    then two weighted stationaries a0 = tentY*w0, a1 = tentY*w1 feed two PE
    matmuls accumulating into slab s and slab s+1 banks. Window-straddling
    y/x tap pairs are duplicated into both windows by the host; the
    window-local tents pick up exactly the in-window taps and out-of-grid
    taps vanish (grid_sample zero-padding semantics).
  - Signed weights accumulate the difference volume d directly; each PSUM
    tile sees one contiguous matmul accumulation group (mandatory on HW),
    and slab evacuation sums the slab's two phase-tiles and computes fused
    Huber partial sums Huber(d) = 0.5*|d|^2 - 0.5*relu(|d|-1)^2 via
    activation accumulate.
  - Host sums the 8 cores' [128, 64] partial-sum tiles.
"""

import sys

sys.path.insert(0, "/opt/trn_rl_repo")

import numpy as np
import ml_dtypes

from concourse import bacc, bass, mybir
import concourse.tile as tile
from concourse.bass_utils import run_bass_kernel_spmd

GRID = 256
CORES = 8
SLABS = 32  # slabs per core
YHALVES = 2
XHALVES = 2
NQ = YHALVES * XHALVES  # 4 (yh, xh) combos per slab position
NB0 = 67  # batches per group at slab position 0 (receives straddle halves)
NBI = 35  # batches per group at interior slab positions
NBS = [NB0] + [NBI] * (SLABS - 1)
TOT = NQ * sum(NBS)  # total batch-columns per core (4992)

F32 = mybir.dt.float32
BF16 = mybir.dt.bfloat16


def _group_col_offsets():
    offs = np.zeros(SLABS * NQ, dtype=np.int64)
    col = 0
    for s in range(SLABS):
        for q in range(NQ):
            offs[s * NQ + q] = col
            col += NBS[s]
    assert col == TOT
    return offs


def _prepare_shards(registration_pred, registration_gt, coords):
    """Build per-core [128, TOT] f32 arrays Y, X, W0, W1 of z-pair records."""
    ys, xs, w0s, w1s, bins = [], [], [], [], []
    for vol, reg in ((0, registration_pred), (1, registration_gt)):
        p = coords.astype(np.float32) + reg.astype(np.float32)
        # mirror the reference's exact f32 expression ((g+1)*size - 1) * 0.5
        g = ((p + np.float32(1.0)) * np.float32(GRID) - np.float32(1.0)) * np.float32(
            0.5
        )
        gx = g[:, 0]
        gy = g[:, 1]
        gz = g[:, 2]
        z0 = np.floor(gz)
        fz = (gz - z0).astype(np.float32)
        z0 = z0.astype(np.int64)
        sign = np.float32(1.0 if vol == 0 else -1.0)
        w0 = (1.0 - fz).astype(np.float32) * sign
        w1 = fz * sign
        # z0 == -1: only the z=0 tap is valid -> shift record to z0=0
        shift = z0 == -1
        w0 = np.where(shift, fz * sign, w0)
        w1 = np.where(shift, 0.0, w1)
        z0 = np.where(shift, 0, z0)
        # z0 == 255: the z=256 tap is out of grid
        w1 = np.where(z0 == GRID - 1, 0.0, w1)
        keep = (z0 >= 0) & (z0 <= GRID - 1)
        z0k, gyk, gxk = z0[keep], gy[keep], gx[keep]
        w0k, w1k = w0[keep], w1[keep]
        # split records whose taps straddle a core boundary
        strad = ((z0k % SLABS) == SLABS - 1) & (z0k < GRID - 1)
        w1a = np.where(strad, 0.0, w1k)
        recs = [
            (z0k, gyk, gxk, w0k, w1a),
            (z0k[strad] + 1, gyk[strad], gxk[strad], w1k[strad], np.zeros(strad.sum(), np.float32)),
        ]
        for zz, gyv, gxv, rw0, rw1 in recs:
            y0 = np.floor(gyv)
            x0 = np.floor(gxv)
            yh = np.clip(y0 // 128, 0, 1).astype(np.int64)
            xh = np.clip(x0 // 128, 0, 1).astype(np.int64)
            # duplicate window-straddling y/x tap pairs into the upper window
            dupy = y0 == 127
            dupx = x0 == 127
            dupyx = dupy & dupx
            for sel, byh, bxh in (
                (slice(None), yh, xh),
                (dupy, 1, xh[dupy]),
                (dupx, yh[dupx], 1),
                (dupyx, 1, 1),
            ):
                ys.append(gyv[sel])
                xs.append(gxv[sel])
                w0s.append(rw0[sel])
                w1s.append(rw1[sel])
                bins.append((zz[sel] * 2 + byh) * 2 + bxh)
    Y = np.concatenate(ys)
    X = np.concatenate(xs)
    W0 = np.concatenate(w0s)
    W1 = np.concatenate(w1s)
    B = np.concatenate(bins)  # global bin in [0, 1024)

    order = np.argsort(B, kind="stable")
    Y, X, W0, W1, B = Y[order], X[order], W0[order], W1[order], B[order]
    nbins = GRID * NQ
    counts = np.bincount(B, minlength=nbins)

    offs_core = _group_col_offsets()  # per (slab_pos, q) within-core col offset
    zz = np.arange(GRID)
    core_of = zz // SLABS
    pos_of = zz % SLABS
    bin_caps = np.repeat(np.array(NBS)[pos_of] * 128, NQ)
    if (counts > bin_caps).any():
        raise RuntimeError("bin overflow")
    # global column offset per bin
    bin_cols = (
        core_of.repeat(NQ) * TOT
        + offs_core[(pos_of.repeat(NQ) * NQ) + np.tile(np.arange(NQ), GRID)]
    )

    starts = np.zeros(nbins + 1, dtype=np.int64)
    np.cumsum(counts, out=starts[1:])
    rank = np.arange(len(B), dtype=np.int64) - starts[B]
    dest = bin_cols[B] * 128 + rank

    def field_tiles(vals):
        flat = np.zeros(CORES * TOT * 128, dtype=np.float32)
        flat[dest] = vals
        out = []
        for c in range(CORES):
            block = flat[c * TOT * 128 : (c + 1) * TOT * 128]
            out.append(np.ascontiguousarray(block.reshape(TOT, 128).T))
        return out

    return list(
        zip(field_tiles(Y), field_tiles(X), field_tiles(W0), field_tiles(W1))
    )


def _sb_chunks(nb):
    """Split nb batches into superblocks of up to 16."""
    out = []
    i = 0
    while i < nb:
        sz = min(16, nb - i)
        out.append((i, sz))
        i += sz
    return out


def _build_program():
    nc = bacc.Bacc("TRN2", target_bir_lowering=False, debug=False, num_devices=CORES)
    Yd = nc.declare_dram_parameter("Y", [128, TOT], F32, isOutput=False)
    Xd = nc.declare_dram_parameter("X", [128, TOT], F32, isOutput=False)
    W0d = nc.declare_dram_parameter("W0", [128, TOT], F32, isOutput=False)
    W1d = nc.declare_dram_parameter("W1", [128, TOT], F32, isOutput=False)
    IOTAd = nc.declare_dram_parameter("IOTA", [128, GRID], BF16, isOutput=False)
    OUTd = nc.declare_dram_parameter("OUT", [128, 2 * SLABS], F32, isOutput=True)

    AluOp = mybir.AluOpType
    Act = mybir.ActivationFunctionType
    offs_core = _group_col_offsets()

    with tile.TileContext(nc) as tc:
        with (
            tc.tile_pool(name="persist", bufs=1) as persist,
            tc.tile_pool(name="eab", bufs=4) as eab,
            tc.tile_pool(name="tab", bufs=3) as tab,
            tc.tile_pool(name="atile", bufs=12) as atile,
            tc.tile_pool(name="evac", bufs=2) as evac,
            tc.tile_pool(name="psum", bufs=8, space="PSUM") as psum,
        ):
            y_t = persist.tile([128, TOT], F32, tag="yt")
            nc.sync.dma_start(out=y_t[:], in_=Yd[:])
            x_t = persist.tile([128, TOT], F32, tag="xt")
            nc.sync.dma_start(out=x_t[:], in_=Xd[:])
            w0_t = persist.tile([128, TOT], F32, tag="w0t")
            nc.sync.dma_start(out=w0_t[:], in_=W0d[:])
            w1_t = persist.tile([128, TOT], F32, tag="w1t")
            nc.sync.dma_start(out=w1_t[:], in_=W1d[:])
            iota_t = persist.tile([128, GRID], BF16, tag="iota")
            nc.sync.dma_start(out=iota_t[:], in_=IOTAd[:])
            xn_t = persist.tile([128, TOT], F32, tag="xnt")
            nc.vector.tensor_scalar(
                out=xn_t[:],
                in0=x_t[:],
                scalar1=-1.0,
                scalar2=None,
                op0=AluOp.mult,
            )
            acc_u = persist.tile([128, SLABS], F32, tag="accu")
            acc_r = persist.tile([128, SLABS], F32, tag="accr")
            negone = persist.tile([128, 1], F32, tag="negone")
            nc.gpsimd.memset(negone[:], -1.0)

            # Each z0-group s writes two tile-pairs: cur (slab s, w0 taps) and
            # nxt (slab s+1, w1 taps). Each tile sees one contiguous PSUM
            # accumulation group; slab s's total = cur(s) + nxt from group
            # s-1, summed during evacuation.
            prev = None
            for s in range(SLABS):
                cur = psum.tile([128, 512], F32, tag="bank")
                nxt = psum.tile([128, 512], F32, tag="bank")
                nb = NBS[s]
                for gy in range(YHALVES):
                    for gx in range(XHALVES):
                        if True:
                            qq = gy * XHALVES + gx
                            base = offs_core[s * NQ + qq]
                            cr = cur[:, qq * 128 : (qq + 1) * 128]
                            nr = nxt[:, qq * 128 : (qq + 1) * 128]
                            for sb0, sbn in _sb_chunks(nb):
                                da = eab.tile([128, 16 * 128], BF16, tag="da")
                                eb = eab.tile([128, 16 * 128], BF16, tag="eb")
                                for j in range(sbn):
                                    c = base + sb0 + j
                                    nc.vector.tensor_scalar(
                                        out=da[:, j * 128 : (j + 1) * 128],
                                        in0=iota_t[:, gy * 128 : gy * 128 + 128],
                                        scalar1=y_t[:, c : c + 1],
                                        scalar2=None,
                                        op0=AluOp.subtract,
                                    )
                                    # |iota - xc| in one ACT op (bias = -xc)
                                    nc.scalar.activation(
                                        out=eb[:, j * 128 : (j + 1) * 128],
                                        in_=iota_t[:, gx * 128 : gx * 128 + 128],
                                        func=Act.Abs,
                                        bias=xn_t[:, c : c + 1],
                                        scale=1.0,
                                    )
                                ea = eab.tile([128, 16 * 128], BF16, tag="ea")
                                nc.scalar.activation(
                                    out=ea[:, : sbn * 128],
                                    in_=da[:, : sbn * 128],
                                    func=Act.Abs,
                                    bias=0.0,
                                    scale=1.0,
                                )
                                ta = tab.tile([128, 16 * 128], BF16, tag="ta")
                                tb = tab.tile([128, 16 * 128], BF16, tag="tb")
                                # negated tents min(|t|-1, 0); negations cancel
                                nc.vector.tensor_scalar(
                                    out=ta[:, : sbn * 128],
                                    in0=ea[:, : sbn * 128],
                                    scalar1=1.0,
                                    scalar2=0.0,
                                    op0=AluOp.subtract,
                                    op1=AluOp.min,
                                )
                                nc.vector.tensor_scalar(
                                    out=tb[:, : sbn * 128],
                                    in0=eb[:, : sbn * 128],
                                    scalar1=1.0,
                                    scalar2=0.0,
                                    op0=AluOp.subtract,
                                    op1=AluOp.min,
                                )
                                for j in range(sbn):
                                    c = base + sb0 + j
                                    first = sb0 + j == 0
                                    last = sb0 + j == nb - 1
                                    a0 = atile.tile([128, 128], BF16, tag="a0")
                                    nc.vector.tensor_scalar(
                                        out=a0[:],
                                        in0=ta[:, j * 128 : (j + 1) * 128],
                                        scalar1=w0_t[:, c : c + 1],
                                        scalar2=None,
                                        op0=AluOp.mult,
                                    )
                                    nc.tensor.matmul(
                                        cr,
                                        a0[:],
                                        tb[:, j * 128 : (j + 1) * 128],
                                        start=first,
                                        stop=last,
                                    )
                                    a1 = atile.tile([128, 128], BF16, tag="a1")
                                    nc.vector.tensor_scalar(
                                        out=a1[:],
                                        in0=ta[:, j * 128 : (j + 1) * 128],
                                        scalar1=w1_t[:, c : c + 1],
                                        scalar2=None,
                                        op0=AluOp.mult,
                                    )
                                    nc.tensor.matmul(
                                        nr,
                                        a1[:],
                                        tb[:, j * 128 : (j + 1) * 128],
                                        start=first,
                                        stop=last,
                                    )
                # evacuate slab s: d = cur(s) + prev-group nxt; then Huber
                d_sb = evac.tile([128, 512], BF16, tag="dsb")
                if prev is None:
                    nc.vector.tensor_copy(out=d_sb[:], in_=cur[:])
                else:
                    pp = evac.tile([128, 512], BF16, tag="pp")
                    nc.vector.tensor_copy(out=pp[:], in_=prev[:])
                    nc.vector.tensor_tensor(
                        out=d_sb[:], in0=cur[:], in1=pp[:], op=AluOp.add
                    )
                u = evac.tile([128, 512], BF16, tag="u")
                nc.vector.scalar_tensor_tensor(
                    out=u[:],
                    in0=d_sb[:],
                    scalar=-1.0,
                    in1=d_sb[:],
                    op0=AluOp.mult,
                    op1=AluOp.max,
                )
                r = evac.tile([128, 512], BF16, tag="r")
                nc.scalar.activation(
                    out=r[:], in_=u[:], func=Act.Relu, bias=negone[:], scale=1.0
                )
                squ = evac.tile([128, 512], BF16, tag="squ")
                nc.scalar.activation(
                    out=squ[:],
                    in_=u[:],
                    func=Act.Square,
                    accum_out=acc_u[:, s : s + 1],
                )
                sqr = evac.tile([128, 512], BF16, tag="sqr")
                nc.scalar.activation(
                    out=sqr[:],
                    in_=r[:],
                    func=Act.Square,
                    accum_out=acc_r[:, s : s + 1],
                )
                prev = nxt
            nc.sync.dma_start(out=OUTd[:, 0:SLABS], in_=acc_u[:])
            nc.sync.dma_start(out=OUTd[:, SLABS : 2 * SLABS], in_=acc_r[:])
    nc.compile()
    return nc


_PROGRAM_CACHE = {}


def _get_program():
    if "nc" not in _PROGRAM_CACHE:
        _PROGRAM_CACHE["nc"] = _build_program()
    return _PROGRAM_CACHE["nc"]


def _iota_input():
    return np.broadcast_to(
        np.arange(GRID, dtype=ml_dtypes.bfloat16)[None, :], (128, GRID)
    ).copy()


def kernel(registration_pred, registration_gt, coords, _trace=False):
    shards = _prepare_shards(registration_pred, registration_gt, coords)
    iota = _iota_input()
    nc = _get_program()
    in_maps = [
        {"Y": y, "X": x, "W0": w0, "W1": w1, "IOTA": iota}
        for (y, x, w0, w1) in shards
    ]
    try:
        res = run_bass_kernel_spmd(nc, in_maps, list(range(CORES)), trace=_trace)
    except Exception:
        # Transient device wedge (e.g. NRT_EXEC_UNIT_UNRECOVERABLE) has been
        # observed to fail a single run and recover on retry.
        res = run_bass_kernel_spmd(nc, in_maps, list(range(CORES)), trace=_trace)
    total = 0.0
    for r in res.results:
        out = r["OUT"].astype(np.float64)
        total += 0.5 * (out[:, :SLABS].sum() - out[:, SLABS:].sum())
    if _trace:
        kernel.last_exec_time_ns = res.exec_time_ns
        kernel.last_results = res
    return np.float32(total)
